# revision 17
# baseline (speedup 1.0000x reference)
"""Trainium2 Bass kernel for nn_DecoderLayer (GNN message passing layer).

Data-parallel over the node axis N=4096 across 8 NeuronCores (512
nodes/core).  v3 design (v1 = 172 us, v2 = 140 us):

- Edge features ship as bf16: HBM traffic halves (DMA active 110 -> 75
  us measured); bf16 and wide-f32r matmuls both run 1 col/cycle so PE
  work is unchanged by the dtype.
- m3 is linear and commutes with the attention multiply and the K-sum,
  so h2*attn is aggregated over K first (DVE) and m3 runs once on the
  [128, 512] aggregate in the dense phase: removes 3x512 PE columns and
  3 PSUM banks per super-block.
- k-major edge layout per super-block (k outer, node inner) makes every
  m1 matmul (3 edge chunks AND the stride-0 node broadcast) a 512-wide
  bank-aligned write (matmul outputs that cross a PSUM bank boundary
  corrupt silently; >512-col outputs are rejected), so gelu1 runs as ONE
  1536-wide ACT instruction per super-block.
- Split emission per iteration: PE queue [m1(t), m2(t-1)], ACT queue
  [gelu2(t-1) x3, gelu1(t)].  m2 trails m1 by a full super-block so no
  in-order PE stall waits on ACT; gelu2 slices run while ACT would
  otherwise idle, so the slps pool rotation (2 banks) never blocks m2.
- PSUM: ps1 [128,1536] x2 bufs (6 banks) + shared [128,512] x2 (2
  banks) = exactly 8 banks.
- Dense phase fully batched at [128, 512]: all four 128-node chunks go
  through m3+residual+LN1+MLP+LN2+mask as ONE wave of wide ops (the v2
  per-chunk version spent 44 us on serial cross-engine latency).
  Per-chunk work only where the partition dim forces it (PE transposes,
  4 per trip, written into one PSUM bank at 128-col offsets).
- ldw-opt stays OFF: walrus rejects bf16 LDWEIGHTS under that pass.
- tensor_tensor_reduce crashes the device (bisected); LN variance uses
  the baseline-proven ACT square+accum_out.
"""

import numpy as np
from contextlib import ExitStack

import ml_dtypes

import concourse.bacc as bacc
import concourse.tile as tile
from concourse import mybir
from concourse._compat import with_exitstack
from concourse.bass_utils import run_bass_kernel_spmd

F32 = mybir.dt.float32
F32R = mybir.dt.float32r
BF16 = mybir.dt.bfloat16
GELU = mybir.ActivationFunctionType.Gelu
IDENT = mybir.ActivationFunctionType.Identity
SQRT = mybir.ActivationFunctionType.Sqrt
SQUARE = mybir.ActivationFunctionType.Square
ADD = mybir.AluOpType.add
SUB = mybir.AluOpType.subtract
MULT = mybir.AluOpType.mult
AXX = mybir.AxisListType.X

# Problem constants
N, K, C, ECTX, HID = 4096, 48, 128, 384, 512
NCORES = 8
NN = N // NCORES            # nodes per core = 512
R = NN * K                  # edge rows per core = 24576
SBN = 32                    # nodes per super-block
SBR = SBN * K               # rows per super-block = 1536
NSB = NN // SBN             # super-blocks per core = 16
EPS = 1e-5
SCALE = 30.0
BF = np.dtype(ml_dtypes.bfloat16)


@with_exitstack
def _decoder_kernel(ctx: ExitStack, tc: tile.TileContext, aps: dict):
    nc = tc.nc

    consts = ctx.enter_context(tc.tile_pool(name="consts", bufs=1))
    ps1p = ctx.enter_context(tc.tile_pool(name="ps1p", bufs=2, space="PSUM"))
    slps = ctx.enter_context(tc.tile_pool(name="slps", bufs=2, space="PSUM"))
    epool = ctx.enter_context(tc.tile_pool(name="epool", bufs=6))
    abpool = ctx.enter_context(tc.tile_pool(name="abpool", bufs=3))
    a1pool = ctx.enter_context(tc.tile_pool(name="a1pool", bufs=3))
    h1pool = ctx.enter_context(tc.tile_pool(name="h1pool", bufs=3))
    h2pool = ctx.enter_context(tc.tile_pool(name="h2pool", bufs=3))
    hapool = ctx.enter_context(tc.tile_pool(name="hapool", bufs=3))
    dpool = ctx.enter_context(tc.tile_pool(name="dpool", bufs=1))
    small = ctx.enter_context(tc.tile_pool(name="small", bufs=2))

    edges = aps["edges"]
    st = {}

    def dma_edges(t):
        # three per-chunk transfers: subrange dep-tracking lets m1's c0
        # matmuls start as soon as the first 1/3 lands, and the pieces
        # pipeline across DMA engines
        eT = epool.tile([128, 3 * SBR], BF16, tag="eT")
        for h in range(6):
            nc.sync.dma_start(
                eT[:, h * (SBR // 2):(h + 1) * (SBR // 2)],
                edges[:, t * 3 * SBR + h * (SBR // 2):
                       t * 3 * SBR + (h + 1) * (SBR // 2)])
        st.setdefault(t, {})["eT"] = eT

    def load_const(name, shape, dtype):
        t = consts.tile(shape, dtype, tag=name)
        nc.sync.dma_start(t[:], aps[name][:])
        return t

    # minimal consts for super-block 0 go first on the sync queue, then the
    # edge stream starts; everything else loads behind edges(0..1).
    w1e = load_const("w1e", [128, 3, 128], BF16)
    w1n = load_const("w1n", [128, 128], BF16)
    b1c = load_const("b1c", [128, 1], F32)
    node_r = load_const("node_r", [128, NN], BF16)
    dma_edges(0)
    w2 = load_const("w2", [128, 128], BF16)
    b2c = load_const("b2c", [128, 1], F32)
    dma_edges(1)
    w3 = load_const("w3", [128, 128], F32R)
    wd1 = load_const("wd1", [128, HID], F32R)
    wd2 = load_const("wd2", [128, 4, 128], F32R)
    b3r = load_const("b3r", [1, 128], F32R)
    bd1 = load_const("bd1", [128, 4], F32)
    bd2 = load_const("bd2", [128, 1], F32)
    g1r4 = load_const("g1r4", [128, 512], F32)
    be1r4 = load_const("be1r4", [128, 512], F32)
    g2r4 = load_const("g2r4", [128, 512], F32)
    be2r4 = load_const("be2r4", [128, 512], F32)
    ident = load_const("ident", [128, 128], F32)
    node_t = load_const("node_t", [128, NN], F32)
    sum_a = load_const("sum_a", [1, NN], F32R)
    mask_t = load_const("mask_t", [128, 4], F32)
    dma_edges(2)

    agg_red = consts.tile([128, NN], F32R, tag="agg_red")
    eps_c = consts.tile([128, 1], F32, tag="eps_c")
    nc.vector.memset(eps_c[:], float(EPS))

    def make_atb(t):
        # attn piece DMA'd on the gpsimd (SWDGE) queue so the big sync
        # queue carries only the edge stream; broadcast follows in-queue.
        at1 = a1pool.tile([1, SBR], BF16, tag="at1")
        nc.gpsimd.dma_start(at1[:], aps["attn"][:, t * SBR:(t + 1) * SBR])
        atb = abpool.tile([128, SBR], BF16, tag="atb")
        nc.gpsimd.partition_broadcast(atb[:], at1[:])
        st.setdefault(t, {})["atb"] = atb

    def stageB_pe(t):
        """m1: 12x 512-wide matmuls (3 node-broadcast + 9 edge) into one
        [128,1536] PSUM tile."""
        s_ = st[t]
        eT = s_["eT"]
        ps1 = ps1p.tile([128, SBR], F32, tag="ps1")
        nv = node_r[:, t * SBN:(t + 1) * SBN]
        for s in range(3):
            nc.tensor.matmul(
                ps1[:, s * 512:(s + 1) * 512]
                .rearrange("p (k n) -> p k n", n=SBN),
                w1n[:],
                nv.unsqueeze(1).broadcast_to([128, 16, SBN]),
                start=True, stop=False, skip_group_check=True)
        for c in range(3):
            for s in range(3):
                nc.tensor.matmul(
                    ps1[:, s * 512:(s + 1) * 512], w1e[:, c, :],
                    eT[:, c * SBR + s * 512:c * SBR + (s + 1) * 512],
                    start=False, stop=(c == 2), skip_group_check=True)
        s_["ps1"] = ps1

    def stageB_act(t):
        """One 1536-wide gelu over ps1 into bf16 h1."""
        s_ = st[t]
        h1 = h1pool.tile([128, SBR], BF16, tag="h1")
        nc.scalar.activation(h1[:], s_["ps1"][:], GELU, bias=b1c[:, :])
        s_["h1"] = h1

    def stageC(t):
        """m2 (3x 512-wide) with eager per-slice gelu2 into bf16 h2."""
        s_ = st[t]
        h1 = s_["h1"]
        h2 = h2pool.tile([128, SBR], BF16, tag="h2")
        for s in range(3):
            ps2 = slps.tile([128, 512], F32, tag="sl")
            nc.tensor.matmul(ps2[:], w2[:],
                             h1[:, s * 512:(s + 1) * 512],
                             start=True, stop=True)
            nc.scalar.activation(h2[:, s * 512:(s + 1) * 512], ps2[:],
                                 GELU, bias=b2c[:, :])
        s_["h2"] = h2

    def stageD(t):
        """attn multiply (bf16) + strided K-reduce into agg_red."""
        s_ = st[t]
        h2a = hapool.tile([128, SBR], BF16, tag="h2a")
        nc.vector.tensor_tensor(h2a[:], s_["h2"][:], s_["atb"][:], op=MULT)
        with nc.allow_low_precision(reason="f32r accumulate is 32-bit"):
            nc.vector.tensor_reduce(
                agg_red[:, t * SBN:(t + 1) * SBN],
                h2a[:].rearrange("p (k n) -> p n k", n=SBN),
                axis=AXX, op=ADD,
            )
        del st[t]

    # ---- pipelined emission ----
    for t in range(NSB + 1):
        if t < NSB:
            make_atb(t)                  # gpsimd, feeds mult(t)
        if t < NSB:
            stageB_pe(t)                 # PE m1(t)
        if 0 <= t - 1 < NSB:
            stageC(t - 1)                # PE m2(t-1); ACT gelu2(t-1)
        if t < NSB:
            stageB_act(t)                # ACT gelu1(t), after gelu2(t-1)
        if 0 <= t - 1 < NSB:
            stageD(t - 1)                # DVE mult + K-reduce
        if t + 3 < NSB:
            dma_edges(t + 3)

    # ---- dense phase, batched over all 512 nodes ([128, 4x128]) ----
    def transpose4(src, tag):
        """4 per-chunk PE transposes of a [128, 512] f32 tile into ONE
        single-bank PSUM tile at 128-col offsets; returns the psum tile."""
        pst = slps.tile([128, 512], F32, tag="sl")
        for q in range(4):
            nc.tensor.transpose(pst[:, q * 128:(q + 1) * 128],
                                src[:, q * 128:(q + 1) * 128], ident[:])
        return pst

    def ln_batched(x, g_rep, be_rep, out_t):
        """LayerNorm over C=128 for all 4 chunks at once: x is
        [128 nodes, (4 chunks, 128 C)] row-major."""
        x3 = x[:].rearrange("p (q c) -> p q c", c=128)
        mu = small.tile([128, 4], F32, tag="mu")
        nc.vector.tensor_reduce(mu[:], x3, axis=AXX, op=ADD)
        mu_s = small.tile([128, 4], F32, tag="mu_s")
        nc.vector.tensor_scalar_mul(mu_s[:], mu[:], 1.0 / 128.0)
        xc = dpool.tile([128, 512], F32, tag="xc")
        nc.vector.tensor_tensor(
            xc[:].rearrange("p (q c) -> p q c", c=128), x3,
            mu_s[:].unsqueeze(2).broadcast_to([128, 4, 128]), op=SUB)
        sq = dpool.tile([128, 512], F32, tag="sq")
        vs = small.tile([128, 4], F32, tag="vs")
        nc.vector.tensor_tensor(sq[:], xc[:], xc[:], op=MULT)
        nc.vector.tensor_reduce(
            vs[:], sq[:].rearrange("p (q c) -> p q c", c=128),
            axis=AXX, op=ADD)
        sd = small.tile([128, 4], F32, tag="sd")
        nc.scalar.activation(sd[:], vs[:], SQRT, scale=1.0 / 128.0,
                             bias=eps_c[:, :])
        rstd = small.tile([128, 4], F32, tag="rstd")
        nc.vector.reciprocal(rstd[:], sd[:])
        if g_rep is None:
            nc.vector.tensor_tensor(
                out_t[:].rearrange("p (q c) -> p q c", c=128),
                xc[:].rearrange("p (q c) -> p q c", c=128),
                rstd[:].unsqueeze(2).broadcast_to([128, 4, 128]), op=MULT)
            return
        xg = dpool.tile([128, 512], F32, tag="xg")
        nc.vector.tensor_tensor(
            xg[:].rearrange("p (q c) -> p q c", c=128),
            xc[:].rearrange("p (q c) -> p q c", c=128),
            rstd[:].unsqueeze(2).broadcast_to([128, 4, 128]), op=MULT)
        xgg = dpool.tile([128, 512], F32, tag="xgg")
        nc.vector.tensor_tensor(xgg[:], xg[:], g_rep[:], op=MULT)
        nc.vector.tensor_tensor(out_t[:], xgg[:], be_rep[:], op=ADD)

    # m3 on the whole aggregate + b3 outer-product, one PSUM bank
    psx = slps.tile([128, 512], F32, tag="sl")
    nc.tensor.matmul(psx[:], w3[:], agg_red[:], start=True, stop=False)
    nc.tensor.matmul(psx[:], b3r[:], sum_a[:], start=False, stop=True)
    xt1 = dpool.tile([128, 512], F32, tag="xt1")
    nc.vector.tensor_tensor(xt1[:], node_t[:], psx[:], op=ADD)
    # feature-major -> node-major
    pst = transpose4(xt1, "t1")
    x_rm = dpool.tile([128, 512], F32, tag="x_rm")
    nc.scalar.copy(x_rm[:], pst[:])
    x1n = dpool.tile([128, 512], F32, tag="x1n")
    ln_batched(x_rm, None if aps["trivial_affine"] else g1r4,
               be1r4, x1n)
    # node-major -> feature-major for the MLP
    pst2 = transpose4(x1n, "t2")
    x1nT = dpool.tile([128, 512], F32R, tag="x1nT")
    nc.scalar.copy(x1nT[:], pst2[:])
    hds = []
    for j in range(4):
        psd = slps.tile([128, 512], F32, tag="sl")
        nc.tensor.matmul(psd[:], wd1[:, j * 128:(j + 1) * 128], x1nT[:],
                         start=True, stop=True)
        h = dpool.tile([128, 512], F32R, tag=f"hd{j}")
        nc.scalar.activation(h[:], psd[:], GELU, bias=bd1[:, j:j + 1])
        hds.append(h)
    psd2 = slps.tile([128, 512], F32, tag="sl")
    for j in range(4):
        nc.tensor.matmul(psd2[:], wd2[:, j, :], hds[j][:],
                         start=(j == 0), stop=(j == 3))
    dT = dpool.tile([128, 512], F32, tag="dT")
    nc.scalar.activation(dT[:], psd2[:], IDENT, bias=bd2[:, :])
    # residual in node-major + LN2 + mask
    pst3 = transpose4(dT, "t3")
    x2 = dpool.tile([128, 512], F32, tag="x2")
    nc.vector.tensor_tensor(x2[:], x1n[:], pst3[:], op=ADD)
    x2n = dpool.tile([128, 512], F32, tag="x2n")
    ln_batched(x2, None if aps["trivial_affine"] else g2r4,
               be2r4, x2n)
    o_sb = dpool.tile([128, 512], F32, tag="o_sb")
    nc.vector.tensor_tensor(
        o_sb[:].rearrange("p (q c) -> p q c", c=128),
        x2n[:].rearrange("p (q c) -> p q c", c=128),
        mask_t[:].unsqueeze(2).broadcast_to([128, 4, 128]), op=MULT)
    nc.sync.dma_start(
        aps["out"].rearrange("(q p) c -> p q c", q=4),
        o_sb[:].rearrange("p (q c) -> p q c", c=128))


_CACHE = {}


def _build_program(trivial_affine=False):
    key = ("nc", trivial_affine)
    if key in _CACHE:
        return _CACHE[key]
    nc = bacc.Bacc("TRN2", target_bir_lowering=False, debug=False)
    aps = {}

    def din(name, shape, dtype):
        aps[name] = nc.dram_tensor(name, shape, dtype, kind="ExternalInput").ap()

    din("edges", [128, NSB * 3 * SBR], BF16)
    din("attn", [1, R], BF16)
    din("node_t", [128, NN], F32)
    din("node_r", [128, NN], BF16)
    din("sum_a", [1, NN], F32R)
    din("mask_t", [128, 4], F32)
    din("w1e", [128, 3, 128], BF16)
    din("w1n", [128, 128], BF16)
    din("w2", [128, 128], BF16)
    din("w3", [128, 128], F32R)
    din("wd1", [128, HID], F32R)
    din("wd2", [128, 4, 128], F32R)
    din("b1c", [128, 1], F32)
    din("b2c", [128, 1], F32)
    din("b3r", [1, 128], F32R)
    din("bd1", [128, 4], F32)
    din("bd2", [128, 1], F32)
    din("g1r4", [128, 512], F32)
    din("be1r4", [128, 512], F32)
    din("g2r4", [128, 512], F32)
    din("be2r4", [128, 512], F32)
    din("ident", [128, 128], F32)
    aps["out"] = nc.dram_tensor("out", [NN, C], F32, kind="ExternalOutput").ap()
    aps["trivial_affine"] = trivial_affine

    with tile.TileContext(nc) as tc:
        _decoder_kernel(tc, aps)
    nc.compile()
    _CACHE[key] = nc
    return nc


def _prep_shared(W_m1, b_m1, W_m2, b_m2, W_m3, b_m3, g1, beta1,
                 W_d1, b_d1, W_d2, b_d2, g2, beta2):
    f = np.float32
    rep4 = lambda v: np.ascontiguousarray(np.tile(np.asarray(v, f)[None, :],
                                                  (128, 4)))
    return {
        "w1e": np.ascontiguousarray(
            np.asarray(W_m1, f)[:, C:].T.reshape(3, 128, 128)
            .transpose(1, 0, 2)).astype(BF),
        "w1n": np.ascontiguousarray(np.asarray(W_m1, f)[:, :C].T).astype(BF),
        "w2": np.ascontiguousarray(np.asarray(W_m2, f).T).astype(BF),
        "w3": np.ascontiguousarray((np.asarray(W_m3, f) / SCALE).T),
        "wd1": np.ascontiguousarray(np.asarray(W_d1, f).T),
        "wd2": np.ascontiguousarray(
            np.asarray(W_d2, f).T.reshape(4, 128, 128).transpose(1, 0, 2)),
        "b1c": np.ascontiguousarray(np.asarray(b_m1, f)[:, None]),
        "b2c": np.ascontiguousarray(np.asarray(b_m2, f)[:, None]),
        "b3r": np.ascontiguousarray(np.asarray(b_m3, f)[None, :]),
        "bd1": np.ascontiguousarray(np.asarray(b_d1, f).reshape(4, 128).T),
        "bd2": np.ascontiguousarray(np.asarray(b_d2, f)[:, None]),
        "g1r4": rep4(g1), "be1r4": rep4(beta1),
        "g2r4": rep4(g2), "be2r4": rep4(beta2),
        "ident": np.eye(128, dtype=f),
    }


def _prep_core(node_features, layer_edge_features, mask, attention_mask, ci):
    """Per-core input map: k-major bf16 edge interleave + small tensors."""
    f = np.float32
    lo, hi = ci * NN, (ci + 1) * NN
    e = layer_edge_features[lo:hi]                      # [NN, K, ECTX]
    # eT[p, t, c, k, n] = e[t*SBN+n, k, c*128+p]
    edges_il = np.ascontiguousarray(
        e.reshape(NSB, SBN, K, 3, 128).transpose(4, 0, 3, 2, 1)
        .reshape(128, NSB * 3 * SBR)).astype(BF)
    am = attention_mask[lo:hi]                          # [NN, K]
    attn_il = np.ascontiguousarray(
        am.reshape(NSB, SBN, K).transpose(0, 2, 1).reshape(1, R)).astype(BF)
    nt = np.ascontiguousarray(node_features[lo:hi].T)
    return {
        "edges": edges_il,
        "attn": attn_il,
        "node_t": nt,
        "node_r": nt.astype(BF),
        "sum_a": np.ascontiguousarray(
            (am.sum(axis=1) / SCALE).reshape(1, NN).astype(f)),
        "mask_t": np.ascontiguousarray(mask[lo:hi].reshape(4, 128).T),
    }


def kernel(node_features, layer_edge_features, mask, attention_mask,
           W_m1, b_m1, W_m2, b_m2, W_m3, b_m3, g1, beta1,
           W_d1, b_d1, W_d2, b_d2, g2, beta2):
    f = np.float32
    node_features = np.asarray(node_features, f)
    layer_edge_features = np.asarray(layer_edge_features, f)
    mask = np.asarray(mask, f)
    attention_mask = np.asarray(attention_mask, f)

    shared = _prep_shared(W_m1, b_m1, W_m2, b_m2, W_m3, b_m3, g1, beta1,
                          W_d1, b_d1, W_d2, b_d2, g2, beta2)

    in_maps = []
    for ci in range(NCORES):
        m = _prep_core(node_features, layer_edge_features, mask,
                       attention_mask, ci)
        m.update(shared)
        in_maps.append(m)

    trivial = bool(
        np.all(np.asarray(g1, f) == 1.0) and np.all(np.asarray(beta1, f) == 0.0)
        and np.all(np.asarray(g2, f) == 1.0)
        and np.all(np.asarray(beta2, f) == 0.0))
    nc = _build_program(trivial_affine=trivial)
    res = run_bass_kernel_spmd(nc, in_maps, core_ids=list(range(NCORES)))
    out = np.concatenate([res.results[i]["out"] for i in range(NCORES)], axis=0)
    return out.astype(np.float32)
